# revision 26
# baseline (speedup 1.0000x reference)
"""Multi-head attention kernel for 8 TRN2 NeuronCores.

Problem: B=2, S=2048, D=1024, H=16 heads, head_dim=64, fp32 I/O.

Sharding: 8 cores = 2 batches x 4 head-groups. Core c handles batch c//4 and
heads [4*(c%4), 4*(c%4)+4). Per core: qT/kT projections (feature-on-partition,
2 heads per 128-partition tile), v natural (+ones column), scoresT = k @ q.T
per head (two heads row-tiled in the PE array), exp on ScalarE with scale and
mask/bias folded in, attn@v with the [v|1] trick (softmax denominator on PSUM
row 64), reciprocal + rank-1 PE broadcast normalize, partial out-projection
over the core's 256 features. Host sums 4 partials per batch.

v3 changes over the original baseline (fp8 was tried and REJECTED: any fp8
cast in the value path costs 2.5-5% final error vs the 2e-2 gate):
  - All linear biases folded out of device code exactly (bk cancels in
    softmax; bq enters via the host-computed exp-bias vector mb; bv/bo fold
    into a host-side output offset). Saves DVE work and DMA.
  - Lookahead-1 software pipelining: scores(t+1) is emitted before attnv(t),
    so the PE never sits behind the exp(t) semaphore at block boundaries.
  - Startup: xT s-chunk 0 is split across the GpSimd AND Vector DMA rings
    (the Vector HWDGE ring was unused), and the prefix starts scores after
    only a 256-column kT sliver instead of the full 512-column s-chunk.
  - Tail: the last block normalizes via direct DVE reciprocal (no DMA
    bounce) and the final out-proj chunks alternate their PSUM-evict stage
    between ScalarE and DVE and their DMA between the Sync and Scalar rings.
"""

import numpy as np
import ml_dtypes

import concourse.mybir as mybir
import concourse.tile as tile
from concourse import bacc
from concourse.bass_utils import run_bass_kernel_spmd

BF16 = mybir.dt.bfloat16
FP32 = mybir.dt.float32

B, S, D = 2, 2048, 1024
NH, DH = 16, 64
NCORES = 8
GROUPS = 4                 # head-groups (cores per batch)
HL = NH // GROUPS          # heads per core = 4
FL = HL * DH               # features per core = 256
NPAIR = HL // 2            # head pairs per core = 2

SC = 512                   # i/s chunk (PSUM bank = 512 fp32)
JC = 128                   # j chunk (partition dim)
DCH = D // 128             # contraction chunks over embed dim = 8
N_SC = S // SC             # 4
N_JC = S // JC             # 16


def build_kernel():
    nc = bacc.Bacc("TRN2", target_bir_lowering=False, debug=False)

    xT = nc.dram_tensor("xT", [D, S], BF16, kind="ExternalInput")
    wq = nc.dram_tensor("wq", [128, DCH * FL], BF16, kind="ExternalInput")
    wk = nc.dram_tensor("wk", [128, DCH * FL], BF16, kind="ExternalInput")
    wv = nc.dram_tensor("wv", [128, DCH * FL], BF16, kind="ExternalInput")
    wo = nc.dram_tensor("wo", [128, 2 * D], BF16, kind="ExternalInput")
    mb = nc.dram_tensor("mb", [128, N_JC], FP32, kind="ExternalInput")
    # bf16 partials: halves the 8MB/core output stream; the host-side sum of
    # 4 partials in fp32 keeps the added error at ~0.17% (measured)
    out = nc.dram_tensor("out", [S, D], BF16, kind="ExternalOutput")

    with tile.TileContext(nc) as tc:
        with (
            tc.tile_pool(name="weights", bufs=1) as wpool,
            tc.tile_pool(name="acts", bufs=1) as apool,
            tc.tile_pool(name="exps", bufs=12) as epool,
            tc.tile_pool(name="stages", bufs=6) as spool,
            tc.tile_pool(name="smalls", bufs=6) as smpool,
            tc.tile_pool(name="scores", bufs=2, space="PSUM") as scpool,
            tc.tile_pool(name="attnout", bufs=2, space="PSUM") as aopool,
            tc.tile_pool(name="projacc", bufs=2, space="PSUM") as prpool,
        ):
            # ---- resident inputs ----
            # DMA issue instructions cost ~0.7us (HWDGE) / ~1us (SWDGE) of
            # their queue, and Scalar-ring issues block the ACT queue, so use
            # ONE strided 3D-AP DMA per xT s-chunk and keep the Scalar ring
            # to 4 issues. Assignment is deadline-driven:
            #   sync:   wk, sc0a(cols 0:256), wq, sc3, wo
            #   scalar: wv, mb, sc0b(cols 256:512), sc1
            #   gpsimd: sc2
            # one SBUF tile per DMA region (precise dependency tracking),
            # each laid out (128, DCH, width) and filled by a single strided
            # DMA. s-chunk 0 is two half-tiles so the prefix can start on
            # cols 0:256 while 256:512 is still in flight.
            xtiles = {}   # col0 -> (tile, width)
            for c0, w in [(0, 256), (256, 256), (SC, 256), (SC + 256, 256),
                          (2 * SC, SC), (3 * SC, SC)]:
                xtiles[c0] = (wpool.tile([128, DCH, w], BF16, name=f"xts{c0}"), w)
            xT3 = xT.ap().rearrange("(dc p) s -> p dc s", p=128)

            def xslice(dc, c0, c1):  # absolute columns -> region-tile slice
                for base, (tile_, w) in xtiles.items():
                    if base <= c0 and c1 <= base + w:
                        return tile_[:, dc, c0 - base:c1 - base]
                raise AssertionError(f"xslice span {c0}:{c1} crosses tiles")

            def xchunk(eng, c0, c1):
                tile_, w = xtiles[c0]
                assert c1 - c0 == w
                eng.dma_start(out=tile_, in_=xT3[:, :, c0:c1])

            wk_sb = wpool.tile([128, DCH * FL], BF16, name="wk_sb")
            nc.sync.dma_start(out=wk_sb, in_=wk.ap())
            wkt = [wk_sb[:, dc * FL:(dc + 1) * FL] for dc in range(DCH)]
            xchunk(nc.sync, 0, 256)
            wq_sb = wpool.tile([128, DCH * FL], BF16, name="wq_sb")
            nc.sync.dma_start(out=wq_sb, in_=wq.ap())
            wqt = [wq_sb[:, dc * FL:(dc + 1) * FL] for dc in range(DCH)]
            wo_sb = wpool.tile([128, 2 * D], BF16, name="wo_sb")
            nc.sync.dma_start(out=wo_sb, in_=wo.ap())
            wot = [wo_sb[:, fc * D:(fc + 1) * D] for fc in range(2)]

            wv_sb = wpool.tile([128, DCH * FL], BF16, name="wv_sb")
            nc.scalar.dma_start(out=wv_sb, in_=wv.ap())
            wvt = [wv_sb[:, dc * FL:(dc + 1) * FL] for dc in range(DCH)]
            mb_sb = wpool.tile([128, N_JC], FP32, name="mb_sb")
            nc.scalar.dma_start(out=mb_sb, in_=mb.ap())
            xchunk(nc.scalar, 256, SC)
            xchunk(nc.scalar, SC, SC + 256)
            xchunk(nc.scalar, SC + 256, 2 * SC)

            xchunk(nc.gpsimd, 2 * SC, 3 * SC)
            xchunk(nc.gpsimd, 3 * SC, 4 * SC)

            # ones column at partition 64 for the recip broadcast matmul
            ones65 = wpool.tile([65, 64], BF16, name="ones65")
            nc.vector.memset(ones65[64:65, :], 1.0)
            # warm the ScalarE Exp table set while DMAs stream
            warm = smpool.tile([1, 4], FP32, name="warm", tag="warm")
            nc.vector.memset(warm, 1.0)
            nc.scalar.activation(warm, warm, mybir.ActivationFunctionType.Exp)

            # ---- persistent activations ----
            qt = [apool.tile([128, S], BF16, name=f"qt{p}") for p in range(2)]
            kt = [apool.tile([128, S], BF16, name=f"kt{p}") for p in range(2)]
            vt = [apool.tile([128, HL, 65], BF16, name=f"vt{sc}") for sc in range(N_JC)]
            at = [apool.tile([128, S], BF16, name=f"at{p}") for p in range(2)]

            qk_open = {}

            def qk_cols(dst, w_tiles, fc, c0, c1, dc0, dc1):
                """Columns [c0,c1) of a qT/kT feature tile, d-chunks
                [dc0,dc1). The psum group stays open until dc1==DCH. Ranges
                crossing an x-region-tile boundary become separate psum
                accumulation groups in the same tile."""
                key = (id(dst), fc, c0)
                if dc0 == 0:
                    ps = prpool.tile([128, c1 - c0], FP32, name="ps", tag="ps")
                    qk_open[key] = ps
                else:
                    ps = qk_open[key]
                for dc in range(dc0, dc1):
                    nc.tensor.matmul(
                        ps,
                        lhsT=w_tiles[dc][:, fc * 128:(fc + 1) * 128],
                        rhs=xslice(dc, c0, c1),
                        start=(dc == 0),
                        stop=(dc == DCH - 1),
                    )
                if dc1 == DCH:
                    del qk_open[key]
                    nc.vector.tensor_copy(dst[:, c0:c1], ps)

            def qk_half(dst, w_tiles, sc, fc, half):
                qk_cols(dst, w_tiles, fc, sc * SC, (sc + 1) * SC,
                        half * 4, half * 4 + 4)

            def qk_full(dst, w_tiles, sc, fc):
                qk_half(dst, w_tiles, sc, fc, 0)
                qk_half(dst, w_tiles, sc, fc, 1)

            def v_proj(sc, pair):
                """v rows [128sc,+128) for one head-pair (N=128)."""
                ps = prpool.tile([128, 128], FP32, name="ps", tag="ps")
                for dc in range(DCH):
                    nc.tensor.matmul(
                        ps,
                        lhsT=xslice(dc, sc * JC, (sc + 1) * JC),
                        rhs=wvt[dc][:, pair * 128:(pair + 1) * 128],
                        start=(dc == 0),
                        stop=(dc == DCH - 1),
                    )
                nc.vector.tensor_copy(
                    vt[sc][:, 2 * pair:2 * pair + 2, 0:64],
                    ps.rearrange("p (h d) -> p h d", h=2),
                )
                if pair == 0:
                    nc.vector.memset(vt[sc][:, :, 64:65], 1.0)

            # ---- drip schedule (same slotting discipline as baseline) ----
            K0, Q0, K1, Q1 = (kt[0], wkt, 0), (qt[0], wqt, 0), \
                             (kt[1], wkt, 1), (qt[1], wqt, 1)

            def qk_thunk(args, scn, half):
                dst, w, fc = args
                if scn in (0, 1):
                    # s-chunks 0/1 live in two half-tiles each: split by
                    # columns (full 8-dc group per 256-col half, same
                    # cycles/slot as a dc-half)
                    c0 = scn * SC + half * 256
                    return lambda: qk_cols(dst, w, fc, c0, c0 + 256, 0, DCH)
                return lambda: qk_half(dst, w, scn, fc, half)

            sched = {b: {} for b in range(8)}

            def put(b, jc, thunk):
                sched[b].setdefault(jc, []).append(thunk)

            # block 0: v pair-0 streaming, k0 + q0 sc1
            for j in range(1, N_JC):
                put(0, j - 1, lambda j=j: v_proj(j, 0))
            put(0, 2, qk_thunk(K0, 1, 0)); put(0, 3, qk_thunk(K0, 1, 1))
            put(0, 5, qk_thunk(K0, 2, 0)); put(0, 6, qk_thunk(K0, 2, 1))
            put(0, 9, qk_thunk(K0, 3, 0)); put(0, 10, qk_thunk(K0, 3, 1))
            put(0, 12, qk_thunk(Q0, 1, 0)); put(0, 13, qk_thunk(Q0, 1, 1))
            put(0, 15, qk_thunk(Q0, 2, 0)); put(1, 3, qk_thunk(Q0, 2, 1))
            put(1, 5, qk_thunk(Q0, 3, 0)); put(1, 6, qk_thunk(Q0, 3, 1))
            for i, j in enumerate(range(0, 5)):
                put(1, 7 + i, lambda j=j: v_proj(j, 1))
            put(1, 15, lambda: v_proj(5, 1))
            for i, j in enumerate(range(6, 12)):
                put(2, 3 + 2 * i, lambda j=j: v_proj(j, 1))
            put(2, 15, qk_thunk(K1, 0, 0)); put(3, 3, qk_thunk(K1, 0, 1))
            put(3, 5, qk_thunk(K1, 1, 0)); put(3, 6, qk_thunk(K1, 1, 1))
            put(3, 7, qk_thunk(K1, 2, 0)); put(3, 8, qk_thunk(K1, 2, 1))
            put(3, 9, qk_thunk(K1, 3, 0)); put(3, 10, qk_thunk(K1, 3, 1))
            put(3, 11, qk_thunk(Q1, 0, 0)); put(3, 12, qk_thunk(Q1, 0, 1))
            put(3, 13, lambda: v_proj(12, 1)); put(3, 14, lambda: v_proj(13, 1))
            put(3, 15, qk_thunk(Q1, 1, 0)); put(4, 3, qk_thunk(Q1, 1, 1))
            put(4, 5, lambda: v_proj(14, 1)); put(4, 7, lambda: v_proj(15, 1))
            put(4, 15, qk_thunk(Q1, 2, 0)); put(5, 3, qk_thunk(Q1, 2, 1))
            put(5, 15, qk_thunk(Q1, 3, 0)); put(6, 3, qk_thunk(Q1, 3, 1))

            def out_proj_chunk(ic, ec, ss, stage_eng="v", ring=None):
                srow = ic * SC + ss * 128
                po = prpool.tile([128, SC], FP32, name="po", tag="ps")
                for fc in range(2):
                    nc.tensor.matmul(
                        po,
                        lhsT=at[fc][:, srow:srow + 128],
                        rhs=wot[fc][:, ec * SC:(ec + 1) * SC],
                        start=(fc == 0),
                        stop=(fc == 1),
                    )
                stg = spool.tile([128, SC], BF16, name="ostg")
                if stage_eng == "v":
                    nc.vector.tensor_copy(stg, po)
                else:
                    nc.scalar.copy(stg, po)
                (ring or nc.sync).dma_start(
                    out=out.ap()[srow:srow + 128, ec * SC:(ec + 1) * SC],
                    in_=stg,
                )

            # blocks 5-7: drip previous ic's out_proj (8 chunks each).
            # Slots 7..14: they must follow the deferred t2c at slot 6, which
            # writes the at[64:128] rows these chunks read. DMAs alternate
            # Sync/GpSimd so neither queue backs up (~1MB/block each way).
            for b in range(5, 8):
                ic_prev = b - 5
                idx = 0
                for ec in range(2):
                    for ss in range(SC // 128):
                        put(b, 7 + idx, lambda ic=ic_prev, ec=ec, ss=ss, i=idx:
                            out_proj_chunk(ic, ec, ss,
                                           ring=(nc.sync if i % 2 else nc.gpsimd)))
                        idx += 1

            # ---- the attention pipeline, lookahead-1 on scores ----
            pending_norm = {}
            rec_store = [None]

            steps = [(pair, ic, jc)
                     for pair in range(2) for ic in range(N_SC)
                     for jc in range(N_JC)]
            sc_store = {}
            ao_store = {}

            def emit_scores(t):
                pair, ic, jc = steps[t]
                i_sl = slice(ic * SC, (ic + 1) * SC)
                sc_ps = scpool.tile([128, 2 * SC], FP32, name="sc_ps")
                nc.tensor.matmul(
                    sc_ps[:, 0:SC],
                    lhsT=kt[pair][0:64, jc * JC:(jc + 1) * JC],
                    rhs=qt[pair][0:64, i_sl],
                )
                nc.tensor.matmul(
                    sc_ps[:, SC:2 * SC],
                    lhsT=kt[pair][64:128, jc * JC:(jc + 1) * JC],
                    rhs=qt[pair][64:128, i_sl],
                )
                sc_store[t] = sc_ps

            def normalize(pair, ic, last):
                """Two-stage deferred normalize (mid-kernel) or inline direct
                normalize (last block)."""
                i_sl = slice(ic * SC, (ic + 1) * SC)
                outA, outB = ao_store.pop((pair, ic))
                osbA = smpool.tile([65, SC], FP32, name="osbA", tag="osb")
                nc.vector.tensor_copy(osbA, outA)
                osbB = smpool.tile([65, SC], FP32, name="osbB", tag="osb")
                nc.vector.tensor_copy(osbB, outB)

                def t2a(osbs=(osbA, osbB)):
                    # DMA bounce: reshape (1,512)->(64,8) so the reciprocal
                    # runs at 8 elems/lane, then cast back to a bf16 row.
                    # (Direct (1,512) DVE reciprocal measured 3.3us - the
                    # single-partition path is ~6.5ns/elem. Bounce RT ~1.8us.)
                    out_r = []
                    for osb in osbs:
                        zsp = smpool.tile([64, SC // 64], FP32, name="zsp", tag="zsp")
                        nc.gpsimd.dma_start(out=zsp, in_=osb[64:65, :])
                        rsp = smpool.tile([64, SC // 64], FP32, name="rsp", tag="rsp")
                        nc.vector.reciprocal(rsp, zsp)
                        rec_bf = smpool.tile([65, SC], BF16, name="rec_bf", tag="recbf")
                        nc.gpsimd.dma_start(out=rec_bf[64:65, :], in_=rsp)
                        out_r.append(rec_bf)
                    rec_store[0] = out_r

                def t2b():
                    bc = prpool.tile([64, SC], FP32, name="bc", tag="ps")
                    nc.tensor.matmul(bc, lhsT=ones65[64:65, :], rhs=rec_store[0][0][64:65, :])
                    nc.vector.tensor_mul(at[pair][0:64, i_sl], osbA[0:64, :], bc)

                def t2c():
                    bc = prpool.tile([64, SC], FP32, name="bc", tag="ps")
                    nc.tensor.matmul(bc, lhsT=ones65[64:65, :], rhs=rec_store[0][1][64:65, :])
                    stg = smpool.tile([64, SC], BF16, name="stg", tag="stg")
                    nc.vector.tensor_mul(stg, osbB[0:64, :], bc)
                    # shift to partitions 64..127 (DVE can't cross lanes)
                    nc.sync.dma_start(out=at[pair][64:128, i_sl], in_=stg)

                if last:
                    t2a(); t2b(); t2c()
                else:
                    pending_norm["a"] = t2a
                    pending_norm["b"] = t2b
                    pending_norm["c"] = t2c

            # prefix: kT/qT slivers over the two sc0 half-tiles, vt[0] pair 0
            qk_cols(kt[0], wkt, 0, 0, 256, 0, DCH)
            qk_cols(qt[0], wqt, 0, 0, 256, 0, DCH)
            qk_cols(qt[0], wqt, 0, 256, 512, 0, DCH)
            qk_cols(kt[0], wkt, 0, 256, 512, 0, DCH)
            v_proj(0, 0)

            emit_scores(0)
            for t, (pair, ic, jc) in enumerate(steps):
                block = pair * N_SC + ic
                i_sl = slice(ic * SC, (ic + 1) * SC)
                sc_ps = sc_store.pop(t)
                ex = epool.tile([128, 2 * SC], BF16, name="ex")
                nc.scalar.activation(
                    ex, sc_ps, mybir.ActivationFunctionType.Exp,
                    bias=mb_sb[:, jc:jc + 1], scale=1.0 / np.sqrt(DH),
                )
                # deferred normalize of the PREVIOUS block in fixed slots
                for key, slot in (("a", 1), ("b", 5), ("c", 6)):
                    if jc == slot and key in pending_norm:
                        pending_norm.pop(key)()
                for thunk in sched[block].get(jc, []):
                    thunk()
                # lookahead-1: scores(t+1) go to the PE queue before attnv(t)
                # (and after this slot's drips, whose kt/qt writes they may
                # read), so the PE never idles behind the exp(t) semaphore
                if t + 1 < len(steps):
                    emit_scores(t + 1)
                if jc == 0:
                    outA = aopool.tile([65, SC], FP32, name="outA", tag="ao")
                    outB = aopool.tile([65, SC], FP32, name="outB", tag="ao")
                    ao_store[(pair, ic)] = (outA, outB)
                outA, outB = ao_store[(pair, ic)]
                nc.tensor.matmul(
                    outA, lhsT=vt[jc][:, 2 * pair, :], rhs=ex[:, 0:SC],
                    start=(jc == 0), stop=(jc == N_JC - 1),
                )
                nc.tensor.matmul(
                    outB, lhsT=vt[jc][:, 2 * pair + 1, :], rhs=ex[:, SC:2 * SC],
                    start=(jc == 0), stop=(jc == N_JC - 1),
                )
                if jc == N_JC - 1:
                    normalize(pair, ic, last=(t == len(steps) - 1))

            # tail: final ic's output projection, stages alternating between
            # ScalarE (idle now) and DVE, DMAs alternating Sync/Scalar rings
            rings = [nc.sync, nc.scalar, nc.gpsimd]
            for i, (ec, ss) in enumerate(
                    [(e, s) for s in range(SC // 128) for e in range(2)]):
                out_proj_chunk(N_SC - 1, ec, ss,
                               stage_eng=("v" if i % 2 else "s"),
                               ring=rings[i % 3])

    nc.compile()
    return nc


_NC_CACHE = None


def _get_nc():
    global _NC_CACHE
    if _NC_CACHE is None:
        _NC_CACHE = build_kernel()
    return _NC_CACHE


def make_in_maps(inputs):
    x = np.asarray(inputs["x"], dtype=np.float32)
    mask = np.asarray(inputs["mask"])
    Wq = np.asarray(inputs["Wq"], dtype=np.float32)
    bq = np.asarray(inputs["bq"], dtype=np.float32)
    Wk = np.asarray(inputs["Wk"], dtype=np.float32)
    bk = np.asarray(inputs["bk"], dtype=np.float32)
    Wv = np.asarray(inputs["Wv"], dtype=np.float32)
    Wo = np.asarray(inputs["Wo"], dtype=np.float32)

    bf = ml_dtypes.bfloat16

    def pack_dxf(wT):  # (1024, FL) -> (128, 8*FL): d-chunks side by side
        return np.ascontiguousarray(
            wT.reshape(DCH, 128, FL).transpose(1, 0, 2).reshape(128, DCH * FL)
        )

    def pack_fxe(woT):  # (256, D) -> (128, 2*D): f-chunks side by side
        return np.ascontiguousarray(
            woT.reshape(2, 128, D).transpose(1, 0, 2).reshape(128, 2 * D)
        )

    # bq folds into the exp bias: score_ij += (bq . k_j)/sqrt(dh) with
    # k_j = Wk @ x_j + bk  ->  x_j @ (Wk.T @ bq) + bq.bk  (bk itself shifts
    # scores per-query only and cancels in the softmax)
    wkq = Wk.T @ bq
    bqk = float(bq @ bk)

    in_maps = []
    for c in range(NCORES):
        b = c // GROUPS
        g = c % GROUPS
        fs, fe = g * FL, (g + 1) * FL
        mbias = np.where(mask[b] == 0, np.float32(-1e9), np.float32(0.0))
        mbias = mbias + (x[b] @ wkq + bqk) / np.sqrt(DH)
        in_maps.append({
            "xT": np.ascontiguousarray(x[b].T).astype(bf),
            "wq": pack_dxf(Wq[fs:fe, :].T.astype(bf)),
            "wk": pack_dxf(Wk[fs:fe, :].T.astype(bf)),
            "wv": pack_dxf(Wv[fs:fe, :].T.astype(bf)),
            "wo": pack_fxe(Wo[:, fs:fe].T.astype(bf)),
            "mb": np.ascontiguousarray(
                mbias.astype(np.float32).reshape(N_JC, 128).T
            ),
        })
    return in_maps


def kernel(x, mask, Wq, bq, Wk, bk, Wv, bv, Wo, bo):
    bo = np.asarray(bo, dtype=np.float32)
    bv = np.asarray(bv, dtype=np.float32)
    Wo_np = np.asarray(Wo, dtype=np.float32)
    nc = _get_nc()
    in_maps = make_in_maps(dict(x=x, mask=mask, Wq=Wq, bq=bq, Wk=Wk, bk=bk,
                                Wv=Wv, Wo=Wo))
    res = run_bass_kernel_spmd(nc, in_maps, core_ids=list(range(NCORES)))
    parts = [np.asarray(r["out"], dtype=np.float32) for r in res.results]
    bo_eff = bo + Wo_np @ bv   # bv shifts attn-out by a constant -> Wo @ bv
    full = np.empty((B, S, D), dtype=np.float32)
    for b in range(B):
        acc = parts[b * GROUPS].copy()
        for g in range(1, GROUPS):
            acc += parts[b * GROUPS + g]
        full[b] = acc + bo_eff[None, :]
    return full


# revision 29
# speedup vs baseline: 1.0094x; 1.0094x over previous
"""Multi-head attention kernel for 8 TRN2 NeuronCores.

Problem: B=2, S=2048, D=1024, H=16 heads, head_dim=64, fp32 I/O.

Sharding: 8 cores = 2 batches x 4 head-groups. Core c handles batch c//4 and
heads [4*(c%4), 4*(c%4)+4). Per core: qT/kT projections (feature-on-partition,
2 heads per 128-partition tile), v natural (+ones column), scoresT = k @ q.T
per head (two heads row-tiled in the PE array), exp on ScalarE with scale and
mask/bias folded in, attn@v with the [v|1] trick (softmax denominator on PSUM
row 64), reciprocal + rank-1 PE broadcast normalize, partial out-projection
over the core's 256 features. Host sums 4 partials per batch.

v3 changes over the original baseline (fp8 was tried and REJECTED: any fp8
cast in the value path costs 2.5-5% final error vs the 2e-2 gate):
  - All linear biases folded out of device code exactly (bk cancels in
    softmax; bq enters via the host-computed exp-bias vector mb; bv/bo fold
    into a host-side output offset). Saves DVE work and DMA.
  - Lookahead-1 software pipelining: scores(t+1) is emitted before attnv(t),
    so the PE never sits behind the exp(t) semaphore at block boundaries.
  - Startup: xT s-chunk 0 is split across the GpSimd AND Vector DMA rings
    (the Vector HWDGE ring was unused), and the prefix starts scores after
    only a 256-column kT sliver instead of the full 512-column s-chunk.
  - Tail: the last block normalizes via direct DVE reciprocal (no DMA
    bounce) and the final out-proj chunks alternate their PSUM-evict stage
    between ScalarE and DVE and their DMA between the Sync and Scalar rings.
"""

import numpy as np
import ml_dtypes

import concourse.mybir as mybir
import concourse.tile as tile
from concourse import bacc
from concourse.bass_utils import run_bass_kernel_spmd

BF16 = mybir.dt.bfloat16
FP32 = mybir.dt.float32

B, S, D = 2, 2048, 1024
NH, DH = 16, 64
NCORES = 8
GROUPS = 4                 # head-groups (cores per batch)
HL = NH // GROUPS          # heads per core = 4
FL = HL * DH               # features per core = 256
NPAIR = HL // 2            # head pairs per core = 2

SC = 512                   # i/s chunk (PSUM bank = 512 fp32)
JC = 128                   # j chunk (partition dim)
DCH = D // 128             # contraction chunks over embed dim = 8
N_SC = S // SC             # 4
N_JC = S // JC             # 16


def build_kernel():
    nc = bacc.Bacc("TRN2", target_bir_lowering=False, debug=False)

    xT = nc.dram_tensor("xT", [D, S], BF16, kind="ExternalInput")
    wq = nc.dram_tensor("wq", [128, DCH * FL], BF16, kind="ExternalInput")
    wk = nc.dram_tensor("wk", [128, DCH * FL], BF16, kind="ExternalInput")
    wv = nc.dram_tensor("wv", [128, DCH * FL], BF16, kind="ExternalInput")
    wo = nc.dram_tensor("wo", [128, 2 * D], BF16, kind="ExternalInput")
    mb = nc.dram_tensor("mb", [128, N_JC], FP32, kind="ExternalInput")
    # bf16 partials: halves the 8MB/core output stream; the host-side sum of
    # 4 partials in fp32 keeps the added error at ~0.17% (measured)
    out = nc.dram_tensor("out", [S, D], BF16, kind="ExternalOutput")

    with tile.TileContext(nc) as tc:
        with (
            tc.tile_pool(name="weights", bufs=1) as wpool,
            tc.tile_pool(name="acts", bufs=1) as apool,
            tc.tile_pool(name="exps", bufs=12) as epool,
            tc.tile_pool(name="stages", bufs=6) as spool,
            tc.tile_pool(name="smalls", bufs=6) as smpool,
            tc.tile_pool(name="scores", bufs=2, space="PSUM") as scpool,
            tc.tile_pool(name="attnout", bufs=2, space="PSUM") as aopool,
            tc.tile_pool(name="projacc", bufs=2, space="PSUM") as prpool,
        ):
            # ---- resident inputs ----
            # DMA issue instructions cost ~0.7us (HWDGE) / ~1us (SWDGE) of
            # their queue, and Scalar-ring issues block the ACT queue, so use
            # ONE strided 3D-AP DMA per xT s-chunk and keep the Scalar ring
            # to 4 issues. Assignment is deadline-driven:
            #   sync:   wk, sc0a(cols 0:256), wq, sc3, wo
            #   scalar: wv, mb, sc0b(cols 256:512), sc1
            #   gpsimd: sc2
            # one SBUF tile per DMA region (precise dependency tracking),
            # each laid out (128, DCH, width) and filled by a single strided
            # DMA. s-chunk 0 is two half-tiles so the prefix can start on
            # cols 0:256 while 256:512 is still in flight.
            xtiles = {}   # col0 -> (tile, width)
            for c0, w in [(0, 256), (256, 256), (SC, 256), (SC + 256, 256),
                          (2 * SC, SC), (3 * SC, SC)]:
                xtiles[c0] = (wpool.tile([128, DCH, w], BF16, name=f"xts{c0}"), w)
            xT3 = xT.ap().rearrange("(dc p) s -> p dc s", p=128)

            def xslice(dc, c0, c1):  # absolute columns -> region-tile slice
                for base, (tile_, w) in xtiles.items():
                    if base <= c0 and c1 <= base + w:
                        return tile_[:, dc, c0 - base:c1 - base]
                raise AssertionError(f"xslice span {c0}:{c1} crosses tiles")

            def xchunk(eng, c0, c1):
                tile_, w = xtiles[c0]
                assert c1 - c0 == w
                eng.dma_start(out=tile_, in_=xT3[:, :, c0:c1])

            wk_sb = wpool.tile([128, DCH * FL], BF16, name="wk_sb")
            nc.sync.dma_start(out=wk_sb, in_=wk.ap())
            wkt = [wk_sb[:, dc * FL:(dc + 1) * FL] for dc in range(DCH)]
            xchunk(nc.sync, 0, 256)
            wq_sb = wpool.tile([128, DCH * FL], BF16, name="wq_sb")
            nc.sync.dma_start(out=wq_sb, in_=wq.ap())
            wqt = [wq_sb[:, dc * FL:(dc + 1) * FL] for dc in range(DCH)]
            xchunk(nc.sync, 3 * SC, 4 * SC)
            wo_sb = wpool.tile([128, 2 * D], BF16, name="wo_sb")
            nc.sync.dma_start(out=wo_sb, in_=wo.ap())
            wot = [wo_sb[:, fc * D:(fc + 1) * D] for fc in range(2)]

            wv_sb = wpool.tile([128, DCH * FL], BF16, name="wv_sb")
            nc.scalar.dma_start(out=wv_sb, in_=wv.ap())
            wvt = [wv_sb[:, dc * FL:(dc + 1) * FL] for dc in range(DCH)]
            mb_sb = wpool.tile([128, N_JC], FP32, name="mb_sb")
            nc.scalar.dma_start(out=mb_sb, in_=mb.ap())
            xchunk(nc.scalar, 256, SC)
            xchunk(nc.scalar, SC, SC + 256)
            xchunk(nc.scalar, SC + 256, 2 * SC)

            xchunk(nc.gpsimd, 2 * SC, 3 * SC)

            # ones row at partition 64 for the recip broadcast matmul (widened
            # to 512 so it can also feed the PE warm-up below)
            ones65 = wpool.tile([65, SC], BF16, name="ones65")
            nc.vector.memset(ones65[64:65, :], 1.0)
            # warm the ScalarE Exp table set while DMAs stream
            warm = smpool.tile([1, 4], FP32, name="warm", tag="warm")
            nc.vector.memset(warm, 1.0)
            nc.scalar.activation(warm, warm, mybir.ActivationFunctionType.Exp)
            # PE warm-up: dummy rank-1 matmuls keep the PE array busy while
            # the first DMAs land, so the DVFS ramp (low->mid->full over ~3us
            # of continuous execution) completes before the first real
            # projection instead of running it at 1/2 speed. Results are
            # discarded; the psum tile rotates through the attn-out bufs,
            # which see their first real use ~10us later.
            for i in range(16):
                if i % 8 == 0:
                    dumps = aopool.tile([64, SC], FP32, name="dumps", tag="ao")
                nc.tensor.matmul(dumps, lhsT=ones65[64:65, 0:64],
                                 rhs=ones65[64:65, :])

            # ---- persistent activations ----
            qt = [apool.tile([128, S], BF16, name=f"qt{p}") for p in range(2)]
            kt = [apool.tile([128, S], BF16, name=f"kt{p}") for p in range(2)]
            vt = [apool.tile([128, HL, 65], BF16, name=f"vt{sc}") for sc in range(N_JC)]
            at = [apool.tile([128, S], BF16, name=f"at{p}") for p in range(2)]

            qk_open = {}

            def qk_cols(dst, w_tiles, fc, c0, c1, dc0, dc1):
                """Columns [c0,c1) of a qT/kT feature tile, d-chunks
                [dc0,dc1). The psum group stays open until dc1==DCH. Ranges
                crossing an x-region-tile boundary become separate psum
                accumulation groups in the same tile."""
                key = (id(dst), fc, c0)
                if dc0 == 0:
                    ps = prpool.tile([128, c1 - c0], FP32, name="ps", tag="ps")
                    qk_open[key] = ps
                else:
                    ps = qk_open[key]
                for dc in range(dc0, dc1):
                    nc.tensor.matmul(
                        ps,
                        lhsT=w_tiles[dc][:, fc * 128:(fc + 1) * 128],
                        rhs=xslice(dc, c0, c1),
                        start=(dc == 0),
                        stop=(dc == DCH - 1),
                    )
                if dc1 == DCH:
                    del qk_open[key]
                    nc.vector.tensor_copy(dst[:, c0:c1], ps)

            def qk_half(dst, w_tiles, sc, fc, half):
                qk_cols(dst, w_tiles, fc, sc * SC, (sc + 1) * SC,
                        half * 4, half * 4 + 4)

            def qk_full(dst, w_tiles, sc, fc):
                qk_half(dst, w_tiles, sc, fc, 0)
                qk_half(dst, w_tiles, sc, fc, 1)

            def v_proj(sc, pair):
                """v rows [128sc,+128) for one head-pair (N=128)."""
                ps = prpool.tile([128, 128], FP32, name="ps", tag="ps")
                for dc in range(DCH):
                    nc.tensor.matmul(
                        ps,
                        lhsT=xslice(dc, sc * JC, (sc + 1) * JC),
                        rhs=wvt[dc][:, pair * 128:(pair + 1) * 128],
                        start=(dc == 0),
                        stop=(dc == DCH - 1),
                    )
                nc.vector.tensor_copy(
                    vt[sc][:, 2 * pair:2 * pair + 2, 0:64],
                    ps.rearrange("p (h d) -> p h d", h=2),
                )
                if pair == 0:
                    nc.vector.memset(vt[sc][:, :, 64:65], 1.0)

            # ---- drip schedule (same slotting discipline as baseline) ----
            K0, Q0, K1, Q1 = (kt[0], wkt, 0), (qt[0], wqt, 0), \
                             (kt[1], wkt, 1), (qt[1], wqt, 1)

            def qk_thunk(args, scn, half):
                dst, w, fc = args
                if scn in (0, 1):
                    # s-chunks 0/1 live in two half-tiles each: split by
                    # columns (full 8-dc group per 256-col half, same
                    # cycles/slot as a dc-half)
                    c0 = scn * SC + half * 256
                    return lambda: qk_cols(dst, w, fc, c0, c0 + 256, 0, DCH)
                return lambda: qk_half(dst, w, scn, fc, half)

            sched = {b: {} for b in range(8)}

            def put(b, jc, thunk):
                sched[b].setdefault(jc, []).append(thunk)

            # block 0: v pair-0 streaming, k0 + q0 sc1
            for j in range(1, N_JC):
                put(0, j - 1, lambda j=j: v_proj(j, 0))
            put(0, 2, qk_thunk(K0, 1, 0)); put(0, 3, qk_thunk(K0, 1, 1))
            put(0, 5, qk_thunk(K0, 2, 0)); put(0, 6, qk_thunk(K0, 2, 1))
            put(0, 9, qk_thunk(K0, 3, 0)); put(0, 10, qk_thunk(K0, 3, 1))
            put(0, 12, qk_thunk(Q0, 1, 0)); put(0, 13, qk_thunk(Q0, 1, 1))
            put(0, 15, qk_thunk(Q0, 2, 0)); put(1, 3, qk_thunk(Q0, 2, 1))
            put(1, 5, qk_thunk(Q0, 3, 0)); put(1, 6, qk_thunk(Q0, 3, 1))
            for i, j in enumerate(range(0, 5)):
                put(1, 7 + i, lambda j=j: v_proj(j, 1))
            put(1, 15, lambda: v_proj(5, 1))
            for i, j in enumerate(range(6, 12)):
                put(2, 3 + 2 * i, lambda j=j: v_proj(j, 1))
            put(2, 15, qk_thunk(K1, 0, 0)); put(3, 3, qk_thunk(K1, 0, 1))
            put(3, 5, qk_thunk(K1, 1, 0)); put(3, 6, qk_thunk(K1, 1, 1))
            put(3, 7, qk_thunk(K1, 2, 0)); put(3, 8, qk_thunk(K1, 2, 1))
            put(3, 9, qk_thunk(K1, 3, 0)); put(3, 10, qk_thunk(K1, 3, 1))
            put(3, 11, qk_thunk(Q1, 0, 0)); put(3, 12, qk_thunk(Q1, 0, 1))
            put(3, 13, lambda: v_proj(12, 1)); put(3, 14, lambda: v_proj(13, 1))
            put(3, 15, qk_thunk(Q1, 1, 0)); put(4, 3, qk_thunk(Q1, 1, 1))
            put(4, 5, lambda: v_proj(14, 1)); put(4, 7, lambda: v_proj(15, 1))
            put(4, 15, qk_thunk(Q1, 2, 0)); put(5, 3, qk_thunk(Q1, 2, 1))
            put(5, 15, qk_thunk(Q1, 3, 0)); put(6, 3, qk_thunk(Q1, 3, 1))

            def out_proj_chunk(ic, ec, ss, stage_eng="v", ring=None):
                srow = ic * SC + ss * 128
                po = prpool.tile([128, SC], FP32, name="po", tag="ps")
                for fc in range(2):
                    nc.tensor.matmul(
                        po,
                        lhsT=at[fc][:, srow:srow + 128],
                        rhs=wot[fc][:, ec * SC:(ec + 1) * SC],
                        start=(fc == 0),
                        stop=(fc == 1),
                    )
                stg = spool.tile([128, SC], BF16, name="ostg")
                if stage_eng == "v":
                    nc.vector.tensor_copy(stg, po)
                else:
                    nc.scalar.copy(stg, po)
                (ring or nc.sync).dma_start(
                    out=out.ap()[srow:srow + 128, ec * SC:(ec + 1) * SC],
                    in_=stg,
                )

            # blocks 5-7: drip previous ic's out_proj (8 chunks each).
            # Slots 7..14: they must follow the deferred t2c at slot 6, which
            # writes the at[64:128] rows these chunks read. DMAs alternate
            # Sync/GpSimd so neither queue backs up (~1MB/block each way).
            for b in range(5, 8):
                ic_prev = b - 5
                idx = 0
                for ec in range(2):
                    for ss in range(SC // 128):
                        put(b, 7 + idx, lambda ic=ic_prev, ec=ec, ss=ss, i=idx:
                            out_proj_chunk(ic, ec, ss,
                                           ring=(nc.sync if i % 2 else nc.gpsimd)))
                        idx += 1

            # ---- the attention pipeline, lookahead-1 on scores ----
            pending_norm = {}
            rec_store = [None]

            steps = [(pair, ic, jc)
                     for pair in range(2) for ic in range(N_SC)
                     for jc in range(N_JC)]
            sc_store = {}
            ao_store = {}

            def emit_scores(t):
                pair, ic, jc = steps[t]
                i_sl = slice(ic * SC, (ic + 1) * SC)
                sc_ps = scpool.tile([128, 2 * SC], FP32, name="sc_ps")
                nc.tensor.matmul(
                    sc_ps[:, 0:SC],
                    lhsT=kt[pair][0:64, jc * JC:(jc + 1) * JC],
                    rhs=qt[pair][0:64, i_sl],
                )
                nc.tensor.matmul(
                    sc_ps[:, SC:2 * SC],
                    lhsT=kt[pair][64:128, jc * JC:(jc + 1) * JC],
                    rhs=qt[pair][64:128, i_sl],
                )
                sc_store[t] = sc_ps

            def normalize(pair, ic, last):
                """Two-stage deferred normalize (mid-kernel) or inline direct
                normalize (last block)."""
                i_sl = slice(ic * SC, (ic + 1) * SC)
                outA, outB = ao_store.pop((pair, ic))
                osbA = smpool.tile([65, SC], FP32, name="osbA", tag="osb")
                nc.vector.tensor_copy(osbA, outA)
                osbB = smpool.tile([65, SC], FP32, name="osbB", tag="osb")
                nc.vector.tensor_copy(osbB, outB)

                def t2a(osbs=(osbA, osbB)):
                    # DMA bounce: reshape (1,512)->(64,8) so the reciprocal
                    # runs at 8 elems/lane, then cast back to a bf16 row.
                    # (Direct (1,512) DVE reciprocal measured 3.3us - the
                    # single-partition path is ~6.5ns/elem. Bounce RT ~1.8us.)
                    out_r = []
                    for osb in osbs:
                        zsp = smpool.tile([64, SC // 64], FP32, name="zsp", tag="zsp")
                        nc.gpsimd.dma_start(out=zsp, in_=osb[64:65, :])
                        rsp = smpool.tile([64, SC // 64], FP32, name="rsp", tag="rsp")
                        nc.vector.reciprocal(rsp, zsp)
                        rec_bf = smpool.tile([65, SC], BF16, name="rec_bf", tag="recbf")
                        nc.gpsimd.dma_start(out=rec_bf[64:65, :], in_=rsp)
                        out_r.append(rec_bf)
                    rec_store[0] = out_r

                def t2b():
                    bc = prpool.tile([64, SC], FP32, name="bc", tag="ps")
                    nc.tensor.matmul(bc, lhsT=ones65[64:65, 0:64], rhs=rec_store[0][0][64:65, :])
                    nc.vector.tensor_mul(at[pair][0:64, i_sl], osbA[0:64, :], bc)

                def t2c():
                    bc = prpool.tile([64, SC], FP32, name="bc", tag="ps")
                    nc.tensor.matmul(bc, lhsT=ones65[64:65, 0:64], rhs=rec_store[0][1][64:65, :])
                    stg = smpool.tile([64, SC], BF16, name="stg", tag="stg")
                    nc.vector.tensor_mul(stg, osbB[0:64, :], bc)
                    # shift to partitions 64..127 (DVE can't cross lanes)
                    nc.sync.dma_start(out=at[pair][64:128, i_sl], in_=stg)

                if last:
                    t2a(); t2b(); t2c()
                else:
                    pending_norm["a"] = t2a
                    pending_norm["b"] = t2b
                    pending_norm["c"] = t2c

            # prefix: kT/qT slivers over the two sc0 half-tiles, vt[0] pair 0
            qk_cols(kt[0], wkt, 0, 0, 256, 0, DCH)
            qk_cols(qt[0], wqt, 0, 0, 256, 0, DCH)
            qk_cols(qt[0], wqt, 0, 256, 512, 0, DCH)
            qk_cols(kt[0], wkt, 0, 256, 512, 0, DCH)
            v_proj(0, 0)

            emit_scores(0)
            for t, (pair, ic, jc) in enumerate(steps):
                block = pair * N_SC + ic
                i_sl = slice(ic * SC, (ic + 1) * SC)
                sc_ps = sc_store.pop(t)
                ex = epool.tile([128, 2 * SC], BF16, name="ex")
                nc.scalar.activation(
                    ex, sc_ps, mybir.ActivationFunctionType.Exp,
                    bias=mb_sb[:, jc:jc + 1], scale=1.0 / np.sqrt(DH),
                )
                # deferred normalize of the PREVIOUS block in fixed slots
                for key, slot in (("a", 1), ("b", 5), ("c", 6)):
                    if jc == slot and key in pending_norm:
                        pending_norm.pop(key)()
                for thunk in sched[block].get(jc, []):
                    thunk()
                # lookahead-1: scores(t+1) go to the PE queue before attnv(t)
                # (and after this slot's drips, whose kt/qt writes they may
                # read), so the PE never idles behind the exp(t) semaphore
                if t + 1 < len(steps):
                    emit_scores(t + 1)
                if jc == 0:
                    outA = aopool.tile([65, SC], FP32, name="outA", tag="ao")
                    outB = aopool.tile([65, SC], FP32, name="outB", tag="ao")
                    ao_store[(pair, ic)] = (outA, outB)
                outA, outB = ao_store[(pair, ic)]
                nc.tensor.matmul(
                    outA, lhsT=vt[jc][:, 2 * pair, :], rhs=ex[:, 0:SC],
                    start=(jc == 0), stop=(jc == N_JC - 1),
                )
                nc.tensor.matmul(
                    outB, lhsT=vt[jc][:, 2 * pair + 1, :], rhs=ex[:, SC:2 * SC],
                    start=(jc == 0), stop=(jc == N_JC - 1),
                )
                if jc == N_JC - 1:
                    normalize(pair, ic, last=(t == len(steps) - 1))

            # tail: final ic's output projection, stages alternating between
            # ScalarE (idle now) and DVE, DMAs alternating Sync/Scalar rings
            rings = [nc.sync, nc.scalar, nc.gpsimd]
            for i, (ec, ss) in enumerate(
                    [(e, s) for s in range(SC // 128) for e in range(2)]):
                out_proj_chunk(N_SC - 1, ec, ss,
                               stage_eng=("v" if i % 2 else "s"),
                               ring=rings[i % 3])

    nc.compile()
    return nc


_NC_CACHE = None


def _get_nc():
    global _NC_CACHE
    if _NC_CACHE is None:
        _NC_CACHE = build_kernel()
    return _NC_CACHE


def make_in_maps(inputs):
    x = np.asarray(inputs["x"], dtype=np.float32)
    mask = np.asarray(inputs["mask"])
    Wq = np.asarray(inputs["Wq"], dtype=np.float32)
    bq = np.asarray(inputs["bq"], dtype=np.float32)
    Wk = np.asarray(inputs["Wk"], dtype=np.float32)
    bk = np.asarray(inputs["bk"], dtype=np.float32)
    Wv = np.asarray(inputs["Wv"], dtype=np.float32)
    Wo = np.asarray(inputs["Wo"], dtype=np.float32)

    bf = ml_dtypes.bfloat16

    def pack_dxf(wT):  # (1024, FL) -> (128, 8*FL): d-chunks side by side
        return np.ascontiguousarray(
            wT.reshape(DCH, 128, FL).transpose(1, 0, 2).reshape(128, DCH * FL)
        )

    def pack_fxe(woT):  # (256, D) -> (128, 2*D): f-chunks side by side
        return np.ascontiguousarray(
            woT.reshape(2, 128, D).transpose(1, 0, 2).reshape(128, 2 * D)
        )

    # bq folds into the exp bias: score_ij += (bq . k_j)/sqrt(dh) with
    # k_j = Wk @ x_j + bk  ->  x_j @ (Wk.T @ bq) + bq.bk  (bk itself shifts
    # scores per-query only and cancels in the softmax)
    wkq = Wk.T @ bq
    bqk = float(bq @ bk)

    in_maps = []
    for c in range(NCORES):
        b = c // GROUPS
        g = c % GROUPS
        fs, fe = g * FL, (g + 1) * FL
        mbias = np.where(mask[b] == 0, np.float32(-1e9), np.float32(0.0))
        mbias = mbias + (x[b] @ wkq + bqk) / np.sqrt(DH)
        in_maps.append({
            "xT": np.ascontiguousarray(x[b].T).astype(bf),
            "wq": pack_dxf(Wq[fs:fe, :].T.astype(bf)),
            "wk": pack_dxf(Wk[fs:fe, :].T.astype(bf)),
            "wv": pack_dxf(Wv[fs:fe, :].T.astype(bf)),
            "wo": pack_fxe(Wo[:, fs:fe].T.astype(bf)),
            "mb": np.ascontiguousarray(
                mbias.astype(np.float32).reshape(N_JC, 128).T
            ),
        })
    return in_maps


def kernel(x, mask, Wq, bq, Wk, bk, Wv, bv, Wo, bo):
    bo = np.asarray(bo, dtype=np.float32)
    bv = np.asarray(bv, dtype=np.float32)
    Wo_np = np.asarray(Wo, dtype=np.float32)
    nc = _get_nc()
    in_maps = make_in_maps(dict(x=x, mask=mask, Wq=Wq, bq=bq, Wk=Wk, bk=bk,
                                Wv=Wv, Wo=Wo))
    res = run_bass_kernel_spmd(nc, in_maps, core_ids=list(range(NCORES)))
    parts = [np.asarray(r["out"], dtype=np.float32) for r in res.results]
    bo_eff = bo + Wo_np @ bv   # bv shifts attn-out by a constant -> Wo @ bv
    full = np.empty((B, S, D), dtype=np.float32)
    for b in range(B):
        acc = parts[b * GROUPS].copy()
        for g in range(1, GROUPS):
            acc += parts[b * GROUPS + g]
        full[b] = acc + bo_eff[None, :]
    return full


# revision 30
# speedup vs baseline: 1.0493x; 1.0396x over previous
"""Multi-head attention kernel for 8 TRN2 NeuronCores.

Problem: B=2, S=2048, D=1024, H=16 heads, head_dim=64, fp32 I/O.

Sharding: 8 cores = 2 batches x 4 head-groups. Core c handles batch c//4 and
heads [4*(c%4), 4*(c%4)+4). Per core: qT/kT projections (feature-on-partition,
2 heads per 128-partition tile), v natural (+ones column), scoresT = k @ q.T
per head (two heads row-tiled in the PE array), exp on ScalarE with scale and
mask/bias folded in, attn@v with the [v|1] trick (softmax denominator on PSUM
row 64), reciprocal + rank-1 PE broadcast normalize, partial out-projection
over the core's 256 features. Host sums 4 partials per batch.

Changes over the original baseline (fp8 DoubleRow was tried and REJECTED:
any fp8 cast in the value path costs 2.5-5% final error vs the 2e-2 gate;
it halves PE time but cannot pass correctness):
  - All linear biases folded out of device code exactly (bk shifts scores
    per-query only and cancels in softmax; bq enters scores via the
    host-computed exp-bias vector mb; bv/bo fold into a host-side output
    offset). Saves DVE work and DMA.
  - Lookahead-1 software pipelining: scores(t+1) is emitted before attnv(t),
    so the PE does not idle behind the exp(t) semaphore at block boundaries;
    the normalize (t2a recip bounce, t2b/t2c broadcast matmuls) is deferred
    into fixed slots of the next block instead of running inline at the
    boundary.
  - Startup: xT s-chunk 0 is column-split (GpSimd cols 0:256 / Scalar
    256:512) and the prefix starts scores after only a 256-column kT sliver.
  - Tail: bf16 output partials halve the final DMA flush (host sums the 4
    partials in fp32; adds ~0.17% error, measured), and the final out-proj
    chunks alternate their PSUM-evict stage between ScalarE and DVE and
    their DMAs between the Sync and Scalar rings.
"""

import numpy as np
import ml_dtypes

import concourse.mybir as mybir
import concourse.tile as tile
from concourse import bacc
from concourse.bass_utils import run_bass_kernel_spmd

BF16 = mybir.dt.bfloat16
FP32 = mybir.dt.float32

B, S, D = 2, 2048, 1024
NH, DH = 16, 64
NCORES = 8
GROUPS = 4                 # head-groups (cores per batch)
HL = NH // GROUPS          # heads per core = 4
FL = HL * DH               # features per core = 256
NPAIR = HL // 2            # head pairs per core = 2

SC = 512                   # i/s chunk (PSUM bank = 512 fp32)
JC = 128                   # j chunk (partition dim)
DCH = D // 128             # contraction chunks over embed dim = 8
N_SC = S // SC             # 4
N_JC = S // JC             # 16


def build_kernel():
    nc = bacc.Bacc("TRN2", target_bir_lowering=False, debug=False)

    xT = nc.dram_tensor("xT", [D, S], BF16, kind="ExternalInput")
    wq = nc.dram_tensor("wq", [128, DCH * FL], BF16, kind="ExternalInput")
    wk = nc.dram_tensor("wk", [128, DCH * FL], BF16, kind="ExternalInput")
    wv = nc.dram_tensor("wv", [128, DCH * FL], BF16, kind="ExternalInput")
    wo = nc.dram_tensor("wo", [128, 2 * D], BF16, kind="ExternalInput")
    mb = nc.dram_tensor("mb", [128, N_JC], FP32, kind="ExternalInput")
    # bf16 partials: halves the 8MB/core output stream; the host-side sum of
    # 4 partials in fp32 keeps the added error at ~0.17% (measured)
    out = nc.dram_tensor("out", [S, D], BF16, kind="ExternalOutput")

    with tile.TileContext(nc) as tc:
        with (
            tc.tile_pool(name="weights", bufs=1) as wpool,
            tc.tile_pool(name="acts", bufs=1) as apool,
            tc.tile_pool(name="exps", bufs=12) as epool,
            tc.tile_pool(name="stages", bufs=6) as spool,
            tc.tile_pool(name="smalls", bufs=6) as smpool,
            tc.tile_pool(name="scores", bufs=2, space="PSUM") as scpool,
            tc.tile_pool(name="attnout", bufs=2, space="PSUM") as aopool,
            tc.tile_pool(name="projacc", bufs=2, space="PSUM") as prpool,
        ):
            # ---- resident inputs ----
            # s-chunk-major arrival. The prefix needs cols 0:256 of all
            # d-chunks first (the kT sliver), so sc0 is split column-wise:
            # cols 0:256 on GpSimd, cols 256:512 on Scalar. sc1/sc3 follow on
            # Scalar, sc2 on GpSimd; weights ride the Sync ring.
            xt_all = wpool.tile([128, DCH * S], BF16, name="xt_all")
            xt = [xt_all[:, dc * S:(dc + 1) * S] for dc in range(DCH)]
            for dc in range(DCH):
                nc.gpsimd.dma_start(
                    out=xt[dc][:, 0:256],
                    in_=xT.ap()[dc * 128:(dc + 1) * 128, 0:256],
                )
            for dc in range(DCH):
                nc.scalar.dma_start(
                    out=xt[dc][:, 256:SC],
                    in_=xT.ap()[dc * 128:(dc + 1) * 128, 256:SC],
                )
            for dc in range(DCH):
                nc.scalar.dma_start(
                    out=xt[dc][:, SC:2 * SC],
                    in_=xT.ap()[dc * 128:(dc + 1) * 128, SC:2 * SC],
                )
            for dc in range(DCH):
                nc.gpsimd.dma_start(
                    out=xt[dc][:, 2 * SC:3 * SC],
                    in_=xT.ap()[dc * 128:(dc + 1) * 128, 2 * SC:3 * SC],
                )
            for dc in range(DCH):
                nc.scalar.dma_start(
                    out=xt[dc][:, 3 * SC:4 * SC],
                    in_=xT.ap()[dc * 128:(dc + 1) * 128, 3 * SC:4 * SC],
                )

            wk_sb = wpool.tile([128, DCH * FL], BF16, name="wk_sb")
            nc.sync.dma_start(out=wk_sb, in_=wk.ap())
            wkt = [wk_sb[:, dc * FL:(dc + 1) * FL] for dc in range(DCH)]
            wq_sb = wpool.tile([128, DCH * FL], BF16, name="wq_sb")
            nc.sync.dma_start(out=wq_sb, in_=wq.ap())
            wqt = [wq_sb[:, dc * FL:(dc + 1) * FL] for dc in range(DCH)]
            wv_sb = wpool.tile([128, DCH * FL], BF16, name="wv_sb")
            nc.sync.dma_start(out=wv_sb, in_=wv.ap())
            wvt = [wv_sb[:, dc * FL:(dc + 1) * FL] for dc in range(DCH)]
            mb_sb = wpool.tile([128, N_JC], FP32, name="mb_sb")
            nc.sync.dma_start(out=mb_sb, in_=mb.ap())
            wo_sb = wpool.tile([128, 2 * D], BF16, name="wo_sb")
            nc.sync.dma_start(out=wo_sb, in_=wo.ap())
            wot = [wo_sb[:, fc * D:(fc + 1) * D] for fc in range(2)]

            # ones column at partition 64 for the recip broadcast matmul
            ones65 = wpool.tile([65, 64], BF16, name="ones65")
            nc.vector.memset(ones65[64:65, :], 1.0)
            # warm the ScalarE Exp table set while DMAs stream
            warm = smpool.tile([1, 4], FP32, name="warm", tag="warm")
            nc.vector.memset(warm, 1.0)
            nc.scalar.activation(warm, warm, mybir.ActivationFunctionType.Exp)

            # ---- persistent activations ----
            qt = [apool.tile([128, S], BF16, name=f"qt{p}") for p in range(2)]
            kt = [apool.tile([128, S], BF16, name=f"kt{p}") for p in range(2)]
            vt = [apool.tile([128, HL, 65], BF16, name=f"vt{sc}") for sc in range(N_JC)]
            at = [apool.tile([128, S], BF16, name=f"at{p}") for p in range(2)]

            qk_open = {}

            def qk_cols(dst, w_tiles, fc, c0, c1, dc0, dc1):
                """Columns [c0,c1) of a qT/kT feature tile, d-chunks
                [dc0,dc1). The psum group stays open until dc1==DCH."""
                key = (id(dst), fc, c0)
                if dc0 == 0:
                    ps = prpool.tile([128, c1 - c0], FP32, name="ps", tag="ps")
                    qk_open[key] = ps
                else:
                    ps = qk_open[key]
                for dc in range(dc0, dc1):
                    nc.tensor.matmul(
                        ps,
                        lhsT=w_tiles[dc][:, fc * 128:(fc + 1) * 128],
                        rhs=xt[dc][:, c0:c1],
                        start=(dc == 0),
                        stop=(dc == DCH - 1),
                    )
                if dc1 == DCH:
                    del qk_open[key]
                    nc.vector.tensor_copy(dst[:, c0:c1], ps)

            def qk_half(dst, w_tiles, sc, fc, half):
                qk_cols(dst, w_tiles, fc, sc * SC, (sc + 1) * SC,
                        half * 4, half * 4 + 4)

            def qk_full(dst, w_tiles, sc, fc):
                qk_half(dst, w_tiles, sc, fc, 0)
                qk_half(dst, w_tiles, sc, fc, 1)

            def v_proj(sc, pair):
                """v rows [128sc,+128) for one head-pair (N=128)."""
                ps = prpool.tile([128, 128], FP32, name="ps", tag="ps")
                for dc in range(DCH):
                    nc.tensor.matmul(
                        ps,
                        lhsT=xt[dc][:, sc * JC:(sc + 1) * JC],
                        rhs=wvt[dc][:, pair * 128:(pair + 1) * 128],
                        start=(dc == 0),
                        stop=(dc == DCH - 1),
                    )
                nc.vector.tensor_copy(
                    vt[sc][:, 2 * pair:2 * pair + 2, 0:64],
                    ps.rearrange("p (h d) -> p h d", h=2),
                )
                if pair == 0:
                    nc.vector.memset(vt[sc][:, :, 64:65], 1.0)

            # ---- drip schedule (same slotting discipline as baseline) ----
            K0, Q0, K1, Q1 = (kt[0], wkt, 0), (qt[0], wqt, 0), \
                             (kt[1], wkt, 1), (qt[1], wqt, 1)

            def qk_thunk(args, scn, half):
                dst, w, fc = args
                return lambda: qk_half(dst, w, scn, fc, half)

            sched = {b: {} for b in range(8)}

            def put(b, jc, thunk):
                sched[b].setdefault(jc, []).append(thunk)

            # block 0: k0 sliver-2 at slot 0, v pair-0 streaming, k0 + q0 sc1
            put(0, 0, lambda: qk_cols(kt[0], wkt, 0, 256, 512, 0, DCH))
            for j in range(1, N_JC):
                put(0, j - 1, lambda j=j: v_proj(j, 0))
            put(0, 1, qk_thunk(K0, 1, 0)); put(0, 2, qk_thunk(K0, 1, 1))
            put(0, 5, qk_thunk(K0, 2, 0)); put(0, 6, qk_thunk(K0, 2, 1))
            put(0, 9, qk_thunk(K0, 3, 0)); put(0, 10, qk_thunk(K0, 3, 1))
            put(0, 12, qk_thunk(Q0, 1, 0)); put(0, 13, qk_thunk(Q0, 1, 1))
            put(0, 15, qk_thunk(Q0, 2, 0)); put(1, 3, qk_thunk(Q0, 2, 1))
            put(1, 5, qk_thunk(Q0, 3, 0)); put(1, 6, qk_thunk(Q0, 3, 1))
            for i, j in enumerate(range(0, 5)):
                put(1, 7 + i, lambda j=j: v_proj(j, 1))
            put(1, 15, lambda: v_proj(5, 1))
            for i, j in enumerate(range(6, 12)):
                put(2, 3 + 2 * i, lambda j=j: v_proj(j, 1))
            put(2, 15, qk_thunk(K1, 0, 0)); put(3, 3, qk_thunk(K1, 0, 1))
            put(3, 5, qk_thunk(K1, 1, 0)); put(3, 6, qk_thunk(K1, 1, 1))
            put(3, 7, qk_thunk(K1, 2, 0)); put(3, 8, qk_thunk(K1, 2, 1))
            put(3, 9, qk_thunk(K1, 3, 0)); put(3, 10, qk_thunk(K1, 3, 1))
            put(3, 11, qk_thunk(Q1, 0, 0)); put(3, 12, qk_thunk(Q1, 0, 1))
            put(3, 13, lambda: v_proj(12, 1)); put(3, 14, lambda: v_proj(13, 1))
            put(3, 15, qk_thunk(Q1, 1, 0)); put(4, 3, qk_thunk(Q1, 1, 1))
            put(4, 5, lambda: v_proj(14, 1)); put(4, 7, lambda: v_proj(15, 1))
            put(4, 15, qk_thunk(Q1, 2, 0)); put(5, 3, qk_thunk(Q1, 2, 1))
            put(5, 15, qk_thunk(Q1, 3, 0)); put(6, 3, qk_thunk(Q1, 3, 1))

            def out_proj_chunk(ic, ec, ss, stage_eng="v", ring=None):
                srow = ic * SC + ss * 128
                po = prpool.tile([128, SC], FP32, name="po", tag="ps")
                for fc in range(2):
                    nc.tensor.matmul(
                        po,
                        lhsT=at[fc][:, srow:srow + 128],
                        rhs=wot[fc][:, ec * SC:(ec + 1) * SC],
                        start=(fc == 0),
                        stop=(fc == 1),
                    )
                stg = spool.tile([128, SC], BF16, name="ostg")
                if stage_eng == "v":
                    nc.vector.tensor_copy(stg, po)
                else:
                    nc.scalar.copy(stg, po)
                (ring or nc.sync).dma_start(
                    out=out.ap()[srow:srow + 128, ec * SC:(ec + 1) * SC],
                    in_=stg,
                )

            # blocks 5-7: drip previous ic's out_proj (8 chunks each).
            # Slots 7..14: they must follow the deferred t2c at slot 6, which
            # writes the at[64:128] rows these chunks read.
            for b in range(5, 8):
                ic_prev = b - 5
                idx = 0
                for ec in range(2):
                    for ss in range(SC // 128):
                        put(b, 7 + idx, lambda ic=ic_prev, ec=ec, ss=ss:
                            out_proj_chunk(ic, ec, ss))
                        idx += 1

            # ---- the attention pipeline, lookahead-1 on scores ----
            pending_norm = {}
            rec_store = [None]

            steps = [(pair, ic, jc)
                     for pair in range(2) for ic in range(N_SC)
                     for jc in range(N_JC)]
            sc_store = {}
            ao_store = {}

            def emit_scores(t):
                pair, ic, jc = steps[t]
                i_sl = slice(ic * SC, (ic + 1) * SC)
                sc_ps = scpool.tile([128, 2 * SC], FP32, name="sc_ps")
                nc.tensor.matmul(
                    sc_ps[:, 0:SC],
                    lhsT=kt[pair][0:64, jc * JC:(jc + 1) * JC],
                    rhs=qt[pair][0:64, i_sl],
                )
                nc.tensor.matmul(
                    sc_ps[:, SC:2 * SC],
                    lhsT=kt[pair][64:128, jc * JC:(jc + 1) * JC],
                    rhs=qt[pair][64:128, i_sl],
                )
                sc_store[t] = sc_ps

            def normalize(pair, ic, last):
                """Two-stage deferred normalize (mid-kernel) or inline (last
                block)."""
                i_sl = slice(ic * SC, (ic + 1) * SC)
                outA, outB = ao_store.pop((pair, ic))
                osbA = smpool.tile([65, SC], FP32, name="osbA", tag="osb")
                nc.vector.tensor_copy(osbA, outA)
                osbB = smpool.tile([65, SC], FP32, name="osbB", tag="osb")
                nc.vector.tensor_copy(osbB, outB)

                def t2a(osbs=(osbA, osbB)):
                    # DMA bounce: reshape (1,512)->(64,8) so the reciprocal
                    # runs at 8 elems/lane, then cast back to a bf16 row.
                    # (Direct (1,512) DVE reciprocal measured 3.3us - the
                    # single-partition path is ~6.5ns/elem. Bounce RT ~1.8us.)
                    out_r = []
                    for osb in osbs:
                        zsp = smpool.tile([64, SC // 64], FP32, name="zsp", tag="zsp")
                        nc.gpsimd.dma_start(out=zsp, in_=osb[64:65, :])
                        rsp = smpool.tile([64, SC // 64], FP32, name="rsp", tag="rsp")
                        nc.vector.reciprocal(rsp, zsp)
                        rec_bf = smpool.tile([65, SC], BF16, name="rec_bf", tag="recbf")
                        nc.gpsimd.dma_start(out=rec_bf[64:65, :], in_=rsp)
                        out_r.append(rec_bf)
                    rec_store[0] = out_r

                def t2b():
                    bc = prpool.tile([64, SC], FP32, name="bc", tag="ps")
                    nc.tensor.matmul(bc, lhsT=ones65[64:65, :], rhs=rec_store[0][0][64:65, :])
                    nc.vector.tensor_mul(at[pair][0:64, i_sl], osbA[0:64, :], bc)

                def t2c():
                    bc = prpool.tile([64, SC], FP32, name="bc", tag="ps")
                    nc.tensor.matmul(bc, lhsT=ones65[64:65, :], rhs=rec_store[0][1][64:65, :])
                    stg = smpool.tile([64, SC], BF16, name="stg", tag="stg")
                    nc.vector.tensor_mul(stg, osbB[0:64, :], bc)
                    # shift to partitions 64..127 (DVE can't cross lanes)
                    nc.sync.dma_start(out=at[pair][64:128, i_sl], in_=stg)

                if last:
                    t2a(); t2b(); t2c()
                else:
                    pending_norm["a"] = t2a
                    pending_norm["b"] = t2b
                    pending_norm["c"] = t2c

            # prefix: kT sliver (cols 0:256), q0 s-chunk 0 full, vt[0] pair 0
            qk_cols(kt[0], wkt, 0, 0, 256, 0, DCH)
            qk_full(qt[0], wqt, 0, 0)
            v_proj(0, 0)

            emit_scores(0)
            for t, (pair, ic, jc) in enumerate(steps):
                block = pair * N_SC + ic
                i_sl = slice(ic * SC, (ic + 1) * SC)
                if t + 1 < len(steps):
                    emit_scores(t + 1)
                sc_ps = sc_store.pop(t)
                ex = epool.tile([128, 2 * SC], BF16, name="ex")
                nc.scalar.activation(
                    ex, sc_ps, mybir.ActivationFunctionType.Exp,
                    bias=mb_sb[:, jc:jc + 1], scale=1.0 / np.sqrt(DH),
                )
                # deferred normalize of the PREVIOUS block in fixed slots
                for key, slot in (("a", 1), ("b", 5), ("c", 6)):
                    if jc == slot and key in pending_norm:
                        pending_norm.pop(key)()
                for thunk in sched[block].get(jc, []):
                    thunk()
                if jc == 0:
                    outA = aopool.tile([65, SC], FP32, name="outA", tag="ao")
                    outB = aopool.tile([65, SC], FP32, name="outB", tag="ao")
                    ao_store[(pair, ic)] = (outA, outB)
                outA, outB = ao_store[(pair, ic)]
                nc.tensor.matmul(
                    outA, lhsT=vt[jc][:, 2 * pair, :], rhs=ex[:, 0:SC],
                    start=(jc == 0), stop=(jc == N_JC - 1),
                )
                nc.tensor.matmul(
                    outB, lhsT=vt[jc][:, 2 * pair + 1, :], rhs=ex[:, SC:2 * SC],
                    start=(jc == 0), stop=(jc == N_JC - 1),
                )
                if jc == N_JC - 1:
                    normalize(pair, ic, last=(t == len(steps) - 1))

            # tail: final ic's output projection, stages alternating between
            # ScalarE (idle now) and DVE, DMAs alternating Sync/Scalar rings
            for i, (ec, ss) in enumerate(
                    [(e, s) for s in range(SC // 128) for e in range(2)]):
                out_proj_chunk(N_SC - 1, ec, ss,
                               stage_eng=("v" if i % 2 else "s"),
                               ring=(nc.sync if i % 2 else nc.scalar))

    nc.compile()
    return nc


_NC_CACHE = None


def _get_nc():
    global _NC_CACHE
    if _NC_CACHE is None:
        _NC_CACHE = build_kernel()
    return _NC_CACHE


def make_in_maps(inputs):
    x = np.asarray(inputs["x"], dtype=np.float32)
    mask = np.asarray(inputs["mask"])
    Wq = np.asarray(inputs["Wq"], dtype=np.float32)
    bq = np.asarray(inputs["bq"], dtype=np.float32)
    Wk = np.asarray(inputs["Wk"], dtype=np.float32)
    bk = np.asarray(inputs["bk"], dtype=np.float32)
    Wv = np.asarray(inputs["Wv"], dtype=np.float32)
    Wo = np.asarray(inputs["Wo"], dtype=np.float32)

    bf = ml_dtypes.bfloat16

    def pack_dxf(wT):  # (1024, FL) -> (128, 8*FL): d-chunks side by side
        return np.ascontiguousarray(
            wT.reshape(DCH, 128, FL).transpose(1, 0, 2).reshape(128, DCH * FL)
        )

    def pack_fxe(woT):  # (256, D) -> (128, 2*D): f-chunks side by side
        return np.ascontiguousarray(
            woT.reshape(2, 128, D).transpose(1, 0, 2).reshape(128, 2 * D)
        )

    # bq folds into the exp bias: score_ij += (bq . k_j)/sqrt(dh) with
    # k_j = Wk @ x_j + bk  ->  x_j @ (Wk.T @ bq) + bq.bk  (bk itself shifts
    # scores per-query only and cancels in the softmax)
    wkq = Wk.T @ bq
    bqk = float(bq @ bk)

    in_maps = []
    for c in range(NCORES):
        b = c // GROUPS
        g = c % GROUPS
        fs, fe = g * FL, (g + 1) * FL
        mbias = np.where(mask[b] == 0, np.float32(-1e9), np.float32(0.0))
        mbias = mbias + (x[b] @ wkq + bqk) / np.sqrt(DH)
        in_maps.append({
            "xT": np.ascontiguousarray(x[b].T).astype(bf),
            "wq": pack_dxf(Wq[fs:fe, :].T.astype(bf)),
            "wk": pack_dxf(Wk[fs:fe, :].T.astype(bf)),
            "wv": pack_dxf(Wv[fs:fe, :].T.astype(bf)),
            "wo": pack_fxe(Wo[:, fs:fe].T.astype(bf)),
            "mb": np.ascontiguousarray(
                mbias.astype(np.float32).reshape(N_JC, 128).T
            ),
        })
    return in_maps


def kernel(x, mask, Wq, bq, Wk, bk, Wv, bv, Wo, bo):
    bo = np.asarray(bo, dtype=np.float32)
    bv = np.asarray(bv, dtype=np.float32)
    Wo_np = np.asarray(Wo, dtype=np.float32)
    nc = _get_nc()
    in_maps = make_in_maps(dict(x=x, mask=mask, Wq=Wq, bq=bq, Wk=Wk, bk=bk,
                                Wv=Wv, Wo=Wo))
    res = run_bass_kernel_spmd(nc, in_maps, core_ids=list(range(NCORES)))
    parts = [np.asarray(r["out"], dtype=np.float32) for r in res.results]
    bo_eff = bo + Wo_np @ bv   # bv shifts attn-out by a constant -> Wo @ bv
    full = np.empty((B, S, D), dtype=np.float32)
    for b in range(B):
        acc = parts[b * GROUPS].copy()
        for g in range(1, GROUPS):
            acc += parts[b * GROUPS + g]
        full[b] = acc + bo_eff[None, :]
    return full


# revision 31
# speedup vs baseline: 1.0579x; 1.0081x over previous
"""Multi-head attention kernel for 8 TRN2 NeuronCores.

Problem: B=2, S=2048, D=1024, H=16 heads, head_dim=64, fp32 I/O.

Sharding: 8 cores = 2 batches x 4 head-groups. Core c handles batch c//4 and
heads [4*(c%4), 4*(c%4)+4). Per core: qT/kT projections (feature-on-partition,
2 heads per 128-partition tile), v natural (+ones column), scoresT = k @ q.T
per head (two heads row-tiled in the PE array), exp on ScalarE with scale and
mask/bias folded in, attn@v with the [v|1] trick (softmax denominator on PSUM
row 64), reciprocal + rank-1 PE broadcast normalize, partial out-projection
over the core's 256 features. Host sums 4 partials per batch.

Changes over the original baseline (fp8 DoubleRow was tried and REJECTED:
any fp8 cast in the value path costs 2.5-5% final error vs the 2e-2 gate;
it halves PE time but cannot pass correctness):
  - All linear biases folded out of device code exactly (bk shifts scores
    per-query only and cancels in softmax; bq enters scores via the
    host-computed exp-bias vector mb; bv/bo fold into a host-side output
    offset). Saves DVE work and DMA.
  - Lookahead-1 software pipelining: scores(t+1) is emitted before attnv(t),
    so the PE does not idle behind the exp(t) semaphore at block boundaries;
    the normalize (t2a recip bounce, t2b/t2c broadcast matmuls) is deferred
    into fixed slots of the next block instead of running inline at the
    boundary.
  - Startup: xT s-chunk 0 is column-split (GpSimd cols 0:256 / Scalar
    256:512) and the prefix starts scores after only a 256-column kT sliver.
  - Tail: bf16 output partials halve the final DMA flush (host sums the 4
    partials in fp32; adds ~0.17% error, measured), and the final out-proj
    chunks alternate their PSUM-evict stage between ScalarE and DVE and
    their DMAs between the Sync and Scalar rings.
"""

import numpy as np
import ml_dtypes

import concourse.mybir as mybir
import concourse.tile as tile
from concourse import bacc
from concourse.bass_utils import run_bass_kernel_spmd

BF16 = mybir.dt.bfloat16
FP32 = mybir.dt.float32

B, S, D = 2, 2048, 1024
NH, DH = 16, 64
NCORES = 8
GROUPS = 4                 # head-groups (cores per batch)
HL = NH // GROUPS          # heads per core = 4
FL = HL * DH               # features per core = 256
NPAIR = HL // 2            # head pairs per core = 2

SC = 512                   # i/s chunk (PSUM bank = 512 fp32)
JC = 128                   # j chunk (partition dim)
DCH = D // 128             # contraction chunks over embed dim = 8
N_SC = S // SC             # 4
N_JC = S // JC             # 16


def build_kernel():
    nc = bacc.Bacc("TRN2", target_bir_lowering=False, debug=False)

    xT = nc.dram_tensor("xT", [D, S], BF16, kind="ExternalInput")
    wq = nc.dram_tensor("wq", [128, DCH * FL], BF16, kind="ExternalInput")
    wk = nc.dram_tensor("wk", [128, DCH * FL], BF16, kind="ExternalInput")
    wv = nc.dram_tensor("wv", [128, DCH * FL], BF16, kind="ExternalInput")
    wo = nc.dram_tensor("wo", [128, 2 * D], BF16, kind="ExternalInput")
    mb = nc.dram_tensor("mb", [128, N_JC], FP32, kind="ExternalInput")
    # bf16 partials: halves the 8MB/core output stream; the host-side sum of
    # 4 partials in fp32 keeps the added error at ~0.17% (measured)
    out = nc.dram_tensor("out", [S, D], BF16, kind="ExternalOutput")

    with tile.TileContext(nc) as tc:
        with (
            tc.tile_pool(name="weights", bufs=1) as wpool,
            tc.tile_pool(name="acts", bufs=1) as apool,
            tc.tile_pool(name="exps", bufs=12) as epool,
            tc.tile_pool(name="stages", bufs=6) as spool,
            tc.tile_pool(name="smalls", bufs=6) as smpool,
            tc.tile_pool(name="scores", bufs=2, space="PSUM") as scpool,
            tc.tile_pool(name="attnout", bufs=2, space="PSUM") as aopool,
            tc.tile_pool(name="projacc", bufs=2, space="PSUM") as prpool,
        ):
            # ---- resident inputs ----
            # s-chunk-major arrival. The prefix needs cols 0:256 of all
            # d-chunks first (the kT sliver), so sc0 is split column-wise:
            # cols 0:256 on GpSimd, cols 256:512 on Scalar. sc1/sc3 follow on
            # Scalar, sc2 on GpSimd; weights ride the Sync ring.
            xt_all = wpool.tile([128, DCH * S], BF16, name="xt_all")
            xt = [xt_all[:, dc * S:(dc + 1) * S] for dc in range(DCH)]
            for dc in range(DCH):
                nc.gpsimd.dma_start(
                    out=xt[dc][:, 0:256],
                    in_=xT.ap()[dc * 128:(dc + 1) * 128, 0:256],
                )
            for dc in range(DCH):
                nc.scalar.dma_start(
                    out=xt[dc][:, 256:SC],
                    in_=xT.ap()[dc * 128:(dc + 1) * 128, 256:SC],
                )
            for dc in range(DCH):
                nc.scalar.dma_start(
                    out=xt[dc][:, SC:2 * SC],
                    in_=xT.ap()[dc * 128:(dc + 1) * 128, SC:2 * SC],
                )
            for dc in range(DCH):
                nc.gpsimd.dma_start(
                    out=xt[dc][:, 2 * SC:3 * SC],
                    in_=xT.ap()[dc * 128:(dc + 1) * 128, 2 * SC:3 * SC],
                )
            # sc3 rides GpSimd behind sc2: keeping it off the Scalar ring
            # saves 8 x ~0.7us of ACT-queue blockage ahead of the first exp
            for dc in range(DCH):
                nc.gpsimd.dma_start(
                    out=xt[dc][:, 3 * SC:4 * SC],
                    in_=xT.ap()[dc * 128:(dc + 1) * 128, 3 * SC:4 * SC],
                )

            wk_sb = wpool.tile([128, DCH * FL], BF16, name="wk_sb")
            nc.sync.dma_start(out=wk_sb, in_=wk.ap())
            wkt = [wk_sb[:, dc * FL:(dc + 1) * FL] for dc in range(DCH)]
            wq_sb = wpool.tile([128, DCH * FL], BF16, name="wq_sb")
            nc.sync.dma_start(out=wq_sb, in_=wq.ap())
            wqt = [wq_sb[:, dc * FL:(dc + 1) * FL] for dc in range(DCH)]
            wv_sb = wpool.tile([128, DCH * FL], BF16, name="wv_sb")
            nc.sync.dma_start(out=wv_sb, in_=wv.ap())
            wvt = [wv_sb[:, dc * FL:(dc + 1) * FL] for dc in range(DCH)]
            mb_sb = wpool.tile([128, N_JC], FP32, name="mb_sb")
            nc.sync.dma_start(out=mb_sb, in_=mb.ap())
            wo_sb = wpool.tile([128, 2 * D], BF16, name="wo_sb")
            nc.sync.dma_start(out=wo_sb, in_=wo.ap())
            wot = [wo_sb[:, fc * D:(fc + 1) * D] for fc in range(2)]

            # ones column at partition 64 for the recip broadcast matmul
            ones65 = wpool.tile([65, 64], BF16, name="ones65")
            nc.vector.memset(ones65[64:65, :], 1.0)
            # warm the ScalarE Exp table set while DMAs stream
            warm = smpool.tile([1, 4], FP32, name="warm", tag="warm")
            nc.vector.memset(warm, 1.0)
            nc.scalar.activation(warm, warm, mybir.ActivationFunctionType.Exp)

            # ---- persistent activations ----
            qt = [apool.tile([128, S], BF16, name=f"qt{p}") for p in range(2)]
            kt = [apool.tile([128, S], BF16, name=f"kt{p}") for p in range(2)]
            vt = [apool.tile([128, HL, 65], BF16, name=f"vt{sc}") for sc in range(N_JC)]
            at = [apool.tile([128, S], BF16, name=f"at{p}") for p in range(2)]

            qk_open = {}

            def qk_cols(dst, w_tiles, fc, c0, c1, dc0, dc1):
                """Columns [c0,c1) of a qT/kT feature tile, d-chunks
                [dc0,dc1). The psum group stays open until dc1==DCH."""
                key = (id(dst), fc, c0)
                if dc0 == 0:
                    ps = prpool.tile([128, c1 - c0], FP32, name="ps", tag="ps")
                    qk_open[key] = ps
                else:
                    ps = qk_open[key]
                for dc in range(dc0, dc1):
                    nc.tensor.matmul(
                        ps,
                        lhsT=w_tiles[dc][:, fc * 128:(fc + 1) * 128],
                        rhs=xt[dc][:, c0:c1],
                        start=(dc == 0),
                        stop=(dc == DCH - 1),
                    )
                if dc1 == DCH:
                    del qk_open[key]
                    nc.vector.tensor_copy(dst[:, c0:c1], ps)

            def qk_half(dst, w_tiles, sc, fc, half):
                qk_cols(dst, w_tiles, fc, sc * SC, (sc + 1) * SC,
                        half * 4, half * 4 + 4)

            def qk_full(dst, w_tiles, sc, fc):
                qk_half(dst, w_tiles, sc, fc, 0)
                qk_half(dst, w_tiles, sc, fc, 1)

            def v_proj(sc, pair):
                """v rows [128sc,+128) for one head-pair (N=128)."""
                ps = prpool.tile([128, 128], FP32, name="ps", tag="ps")
                for dc in range(DCH):
                    nc.tensor.matmul(
                        ps,
                        lhsT=xt[dc][:, sc * JC:(sc + 1) * JC],
                        rhs=wvt[dc][:, pair * 128:(pair + 1) * 128],
                        start=(dc == 0),
                        stop=(dc == DCH - 1),
                    )
                nc.vector.tensor_copy(
                    vt[sc][:, 2 * pair:2 * pair + 2, 0:64],
                    ps.rearrange("p (h d) -> p h d", h=2),
                )
                if pair == 0:
                    nc.vector.memset(vt[sc][:, :, 64:65], 1.0)

            # ---- drip schedule (same slotting discipline as baseline) ----
            K0, Q0, K1, Q1 = (kt[0], wkt, 0), (qt[0], wqt, 0), \
                             (kt[1], wkt, 1), (qt[1], wqt, 1)

            def qk_thunk(args, scn, half):
                dst, w, fc = args
                return lambda: qk_half(dst, w, scn, fc, half)

            sched = {b: {} for b in range(8)}

            def put(b, jc, thunk):
                sched[b].setdefault(jc, []).append(thunk)

            # block 0: k0 sliver-2 at slot 0, v pair-0 streaming, k0 + q0 sc1
            put(0, 0, lambda: qk_cols(kt[0], wkt, 0, 256, 512, 0, DCH))
            for j in range(1, N_JC):
                put(0, j - 1, lambda j=j: v_proj(j, 0))
            put(0, 1, qk_thunk(K0, 1, 0)); put(0, 2, qk_thunk(K0, 1, 1))
            put(0, 5, qk_thunk(K0, 2, 0)); put(0, 6, qk_thunk(K0, 2, 1))
            put(0, 9, qk_thunk(K0, 3, 0)); put(0, 10, qk_thunk(K0, 3, 1))
            put(0, 12, qk_thunk(Q0, 1, 0)); put(0, 13, qk_thunk(Q0, 1, 1))
            put(0, 15, qk_thunk(Q0, 2, 0)); put(1, 3, qk_thunk(Q0, 2, 1))
            put(1, 5, qk_thunk(Q0, 3, 0)); put(1, 6, qk_thunk(Q0, 3, 1))
            for i, j in enumerate(range(0, 5)):
                put(1, 7 + i, lambda j=j: v_proj(j, 1))
            put(1, 15, lambda: v_proj(5, 1))
            for i, j in enumerate(range(6, 12)):
                put(2, 3 + 2 * i, lambda j=j: v_proj(j, 1))
            put(2, 15, qk_thunk(K1, 0, 0)); put(3, 3, qk_thunk(K1, 0, 1))
            put(3, 5, qk_thunk(K1, 1, 0)); put(3, 6, qk_thunk(K1, 1, 1))
            put(3, 7, qk_thunk(K1, 2, 0)); put(3, 8, qk_thunk(K1, 2, 1))
            put(3, 9, qk_thunk(K1, 3, 0)); put(3, 10, qk_thunk(K1, 3, 1))
            put(3, 11, qk_thunk(Q1, 0, 0)); put(3, 12, qk_thunk(Q1, 0, 1))
            put(3, 13, lambda: v_proj(12, 1)); put(3, 14, lambda: v_proj(13, 1))
            put(3, 15, qk_thunk(Q1, 1, 0)); put(4, 3, qk_thunk(Q1, 1, 1))
            put(4, 5, lambda: v_proj(14, 1)); put(4, 7, lambda: v_proj(15, 1))
            put(4, 15, qk_thunk(Q1, 2, 0)); put(5, 3, qk_thunk(Q1, 2, 1))
            put(5, 15, qk_thunk(Q1, 3, 0)); put(6, 3, qk_thunk(Q1, 3, 1))

            def out_proj_chunk(ic, ec, ss, stage_eng="v", ring=None):
                srow = ic * SC + ss * 128
                po = prpool.tile([128, SC], FP32, name="po", tag="ps")
                for fc in range(2):
                    nc.tensor.matmul(
                        po,
                        lhsT=at[fc][:, srow:srow + 128],
                        rhs=wot[fc][:, ec * SC:(ec + 1) * SC],
                        start=(fc == 0),
                        stop=(fc == 1),
                    )
                stg = spool.tile([128, SC], BF16, name="ostg")
                if stage_eng == "v":
                    nc.vector.tensor_copy(stg, po)
                else:
                    nc.scalar.copy(stg, po)
                (ring or nc.sync).dma_start(
                    out=out.ap()[srow:srow + 128, ec * SC:(ec + 1) * SC],
                    in_=stg,
                )

            # blocks 5-7: drip previous ic's out_proj (8 chunks each).
            # Slots 7..14: they must follow the deferred t2c at slot 6, which
            # writes the at[64:128] rows these chunks read.
            for b in range(5, 8):
                ic_prev = b - 5
                idx = 0
                for ec in range(2):
                    for ss in range(SC // 128):
                        put(b, 7 + idx, lambda ic=ic_prev, ec=ec, ss=ss:
                            out_proj_chunk(ic, ec, ss))
                        idx += 1

            # ---- the attention pipeline, lookahead-1 on scores ----
            pending_norm = {}
            rec_store = [None]

            steps = [(pair, ic, jc)
                     for pair in range(2) for ic in range(N_SC)
                     for jc in range(N_JC)]
            sc_store = {}
            ao_store = {}

            def emit_scores(t):
                pair, ic, jc = steps[t]
                i_sl = slice(ic * SC, (ic + 1) * SC)
                sc_ps = scpool.tile([128, 2 * SC], FP32, name="sc_ps")
                nc.tensor.matmul(
                    sc_ps[:, 0:SC],
                    lhsT=kt[pair][0:64, jc * JC:(jc + 1) * JC],
                    rhs=qt[pair][0:64, i_sl],
                )
                nc.tensor.matmul(
                    sc_ps[:, SC:2 * SC],
                    lhsT=kt[pair][64:128, jc * JC:(jc + 1) * JC],
                    rhs=qt[pair][64:128, i_sl],
                )
                sc_store[t] = sc_ps

            def normalize(pair, ic, last):
                """Two-stage deferred normalize (mid-kernel) or inline (last
                block)."""
                i_sl = slice(ic * SC, (ic + 1) * SC)
                outA, outB = ao_store.pop((pair, ic))
                osbA = smpool.tile([65, SC], FP32, name="osbA", tag="osb")
                nc.vector.tensor_copy(osbA, outA)
                osbB = smpool.tile([65, SC], FP32, name="osbB", tag="osb")
                nc.vector.tensor_copy(osbB, outB)

                def t2a(osbs=(osbA, osbB)):
                    # DMA bounce: reshape (1,512)->(64,8) so the reciprocal
                    # runs at 8 elems/lane, then cast back to a bf16 row.
                    # (Direct (1,512) DVE reciprocal measured 3.3us - the
                    # single-partition path is ~6.5ns/elem. Bounce RT ~1.8us.)
                    out_r = []
                    for osb in osbs:
                        zsp = smpool.tile([64, SC // 64], FP32, name="zsp", tag="zsp")
                        nc.gpsimd.dma_start(out=zsp, in_=osb[64:65, :])
                        rsp = smpool.tile([64, SC // 64], FP32, name="rsp", tag="rsp")
                        nc.vector.reciprocal(rsp, zsp)
                        rec_bf = smpool.tile([65, SC], BF16, name="rec_bf", tag="recbf")
                        nc.gpsimd.dma_start(out=rec_bf[64:65, :], in_=rsp)
                        out_r.append(rec_bf)
                    rec_store[0] = out_r

                def t2b():
                    bc = prpool.tile([64, SC], FP32, name="bc", tag="ps")
                    nc.tensor.matmul(bc, lhsT=ones65[64:65, :], rhs=rec_store[0][0][64:65, :])
                    nc.vector.tensor_mul(at[pair][0:64, i_sl], osbA[0:64, :], bc)

                def t2c():
                    bc = prpool.tile([64, SC], FP32, name="bc", tag="ps")
                    nc.tensor.matmul(bc, lhsT=ones65[64:65, :], rhs=rec_store[0][1][64:65, :])
                    stg = smpool.tile([64, SC], BF16, name="stg", tag="stg")
                    nc.vector.tensor_mul(stg, osbB[0:64, :], bc)
                    # shift to partitions 64..127 (DVE can't cross lanes)
                    nc.sync.dma_start(out=at[pair][64:128, i_sl], in_=stg)

                if last:
                    t2a(); t2b(); t2c()
                else:
                    pending_norm["a"] = t2a
                    pending_norm["b"] = t2b
                    pending_norm["c"] = t2c

            # prefix: kT sliver (cols 0:256), q0 s-chunk 0 full, vt[0] pair 0
            qk_cols(kt[0], wkt, 0, 0, 256, 0, DCH)
            qk_full(qt[0], wqt, 0, 0)
            v_proj(0, 0)

            emit_scores(0)
            for t, (pair, ic, jc) in enumerate(steps):
                block = pair * N_SC + ic
                i_sl = slice(ic * SC, (ic + 1) * SC)
                if t + 1 < len(steps):
                    emit_scores(t + 1)
                sc_ps = sc_store.pop(t)
                ex = epool.tile([128, 2 * SC], BF16, name="ex")
                nc.scalar.activation(
                    ex, sc_ps, mybir.ActivationFunctionType.Exp,
                    bias=mb_sb[:, jc:jc + 1], scale=1.0 / np.sqrt(DH),
                )
                # deferred normalize of the PREVIOUS block in fixed slots
                for key, slot in (("a", 1), ("b", 5), ("c", 6)):
                    if jc == slot and key in pending_norm:
                        pending_norm.pop(key)()
                for thunk in sched[block].get(jc, []):
                    thunk()
                if jc == 0:
                    outA = aopool.tile([65, SC], FP32, name="outA", tag="ao")
                    outB = aopool.tile([65, SC], FP32, name="outB", tag="ao")
                    ao_store[(pair, ic)] = (outA, outB)
                outA, outB = ao_store[(pair, ic)]
                nc.tensor.matmul(
                    outA, lhsT=vt[jc][:, 2 * pair, :], rhs=ex[:, 0:SC],
                    start=(jc == 0), stop=(jc == N_JC - 1),
                )
                nc.tensor.matmul(
                    outB, lhsT=vt[jc][:, 2 * pair + 1, :], rhs=ex[:, SC:2 * SC],
                    start=(jc == 0), stop=(jc == N_JC - 1),
                )
                if jc == N_JC - 1:
                    normalize(pair, ic, last=(t == len(steps) - 1))

            # tail: final ic's output projection, stages alternating between
            # ScalarE (idle now) and DVE, DMAs alternating Sync/Scalar rings
            for i, (ec, ss) in enumerate(
                    [(e, s) for s in range(SC // 128) for e in range(2)]):
                out_proj_chunk(N_SC - 1, ec, ss,
                               stage_eng=("v" if i % 2 else "s"),
                               ring=(nc.sync if i % 2 else nc.scalar))

    nc.compile()
    return nc


_NC_CACHE = None


def _get_nc():
    global _NC_CACHE
    if _NC_CACHE is None:
        _NC_CACHE = build_kernel()
    return _NC_CACHE


def make_in_maps(inputs):
    x = np.asarray(inputs["x"], dtype=np.float32)
    mask = np.asarray(inputs["mask"])
    Wq = np.asarray(inputs["Wq"], dtype=np.float32)
    bq = np.asarray(inputs["bq"], dtype=np.float32)
    Wk = np.asarray(inputs["Wk"], dtype=np.float32)
    bk = np.asarray(inputs["bk"], dtype=np.float32)
    Wv = np.asarray(inputs["Wv"], dtype=np.float32)
    Wo = np.asarray(inputs["Wo"], dtype=np.float32)

    bf = ml_dtypes.bfloat16

    def pack_dxf(wT):  # (1024, FL) -> (128, 8*FL): d-chunks side by side
        return np.ascontiguousarray(
            wT.reshape(DCH, 128, FL).transpose(1, 0, 2).reshape(128, DCH * FL)
        )

    def pack_fxe(woT):  # (256, D) -> (128, 2*D): f-chunks side by side
        return np.ascontiguousarray(
            woT.reshape(2, 128, D).transpose(1, 0, 2).reshape(128, 2 * D)
        )

    # bq folds into the exp bias: score_ij += (bq . k_j)/sqrt(dh) with
    # k_j = Wk @ x_j + bk  ->  x_j @ (Wk.T @ bq) + bq.bk  (bk itself shifts
    # scores per-query only and cancels in the softmax)
    wkq = Wk.T @ bq
    bqk = float(bq @ bk)

    in_maps = []
    for c in range(NCORES):
        b = c // GROUPS
        g = c % GROUPS
        fs, fe = g * FL, (g + 1) * FL
        mbias = np.where(mask[b] == 0, np.float32(-1e9), np.float32(0.0))
        mbias = mbias + (x[b] @ wkq + bqk) / np.sqrt(DH)
        in_maps.append({
            "xT": np.ascontiguousarray(x[b].T).astype(bf),
            "wq": pack_dxf(Wq[fs:fe, :].T.astype(bf)),
            "wk": pack_dxf(Wk[fs:fe, :].T.astype(bf)),
            "wv": pack_dxf(Wv[fs:fe, :].T.astype(bf)),
            "wo": pack_fxe(Wo[:, fs:fe].T.astype(bf)),
            "mb": np.ascontiguousarray(
                mbias.astype(np.float32).reshape(N_JC, 128).T
            ),
        })
    return in_maps


def kernel(x, mask, Wq, bq, Wk, bk, Wv, bv, Wo, bo):
    bo = np.asarray(bo, dtype=np.float32)
    bv = np.asarray(bv, dtype=np.float32)
    Wo_np = np.asarray(Wo, dtype=np.float32)
    nc = _get_nc()
    in_maps = make_in_maps(dict(x=x, mask=mask, Wq=Wq, bq=bq, Wk=Wk, bk=bk,
                                Wv=Wv, Wo=Wo))
    res = run_bass_kernel_spmd(nc, in_maps, core_ids=list(range(NCORES)))
    parts = [np.asarray(r["out"], dtype=np.float32) for r in res.results]
    bo_eff = bo + Wo_np @ bv   # bv shifts attn-out by a constant -> Wo @ bv
    full = np.empty((B, S, D), dtype=np.float32)
    for b in range(B):
        acc = parts[b * GROUPS].copy()
        for g in range(1, GROUPS):
            acc += parts[b * GROUPS + g]
        full[b] = acc + bo_eff[None, :]
    return full
